# revision 1
# baseline (speedup 1.0000x reference)
"""BiCutLoss Trainium2 kernel (8-core data parallel over batch).

Reference semantics (B=16384, L=1024):
    temp[b,j]  = argmax(output[b,j,:])          # 1 iff out1 > out0 (ties -> 0)
    idx[b]     = L if row all-ones else index of last zero
    mask[b,j]  = j <= idx[b]
    r1[b,j]    = -1/log2(j+2)  if labels==1 else (j+1)/alpha
    loss       = sum(output[...,1] * mask * r1) / B

Key restructuring: masked_sum = full_sum - tail_sum, where the tail
(j > idx) is confined to the last W columns whenever each row has a zero
decision in its last W positions. For +-symmetric random data
P(no zero in last W=128) = 2^-128 per row; a per-row flag detects the
(cosmically unlikely / adversarial-only) violation and the host falls
back to an exact numpy evaluation, so the kernel is correct for all
inputs. Benefits: out0 is only read on the window (1/8 of it), and the
compare/scan/mask work runs on [128, W] tiles instead of [128, L].

Full sums, two routes balancing VectorE vs TensorE:
  PE route  (most tiles): ql = out1*lab on DVE; colsum(out1), colsum(ql)
             via ones^T-matmul into PSUM; epilogue dots with Bv / D.
  DVE route (a few tiles): r1 = lab*D + Bv materialized against
             partition-broadcast D/Bv tiles; fused (r1*out1) multiply +
             row-sum accumulation in one scalar_tensor_tensor.
Tail sums mirror the same two routes on the window slice.
Host sums the per-core partials and divides by B.
"""

import threading
from contextlib import ExitStack

import numpy as np

B, L = 16384, 1024
N_CORES = 8
ROWS_PER_CORE = B // N_CORES  # 2048
ALPHA = 0.65
W = 64  # tail window width
DVE_ROUTE_TILES = 5  # tiles whose full-sum runs entirely on VectorE

_compiled = threading.local()


def _reward_rows():
    j = np.arange(L, dtype=np.float64)
    bv = (j + 1.0) / ALPHA
    d = -1.0 / np.log2(j + 2.0) - bv
    return bv.astype(np.float32), d.astype(np.float32)


def _build(rows=ROWS_PER_CORE, num_devices=N_CORES, dve_route_tiles=DVE_ROUTE_TILES):
    import concourse.tile as tile
    from concourse import bacc, mybir

    f32 = mybir.dt.float32
    f16 = mybir.dt.float16
    u8 = mybir.dt.uint8
    Alu = mybir.AluOpType
    Act = mybir.ActivationFunctionType

    n_tiles = rows // 128
    n_dve = min(dve_route_tiles, n_tiles)
    n_pe = n_tiles - n_dve

    nc = bacc.Bacc(
        "TRN2",
        target_bir_lowering=False,
        debug=False,
        enable_asserts=True,
        num_devices=num_devices,
    )

    out1_d = nc.dram_tensor("out1", [rows, L], f32, kind="ExternalInput").ap()
    wpack_d = nc.dram_tensor("wpack", [rows, 2 * W], f32, kind="ExternalInput").ap()
    lab_d = nc.dram_tensor("lab", [rows, L], u8, kind="ExternalInput").ap()
    bv_d = nc.dram_tensor("bv", [1, L], f32, kind="ExternalInput").ap()
    dd_d = nc.dram_tensor("dd", [1, L], f32, kind="ExternalInput").ap()
    # partition-broadcast copies for the DVE route ([128, L], same row repeated)
    bvb_d = nc.dram_tensor("bvb", [128, L], f32, kind="ExternalInput").ap()
    ddb_d = nc.dram_tensor("ddb", [128, L], f32, kind="ExternalInput").ap()
    res_d = nc.dram_tensor("partial", [1, 8], f32, kind="ExternalOutput").ap()
    flag_d = nc.dram_tensor("flags", [128, n_tiles], f32, kind="ExternalOutput").ap()
    accs_d = nc.dram_tensor("accs", [128, 2], f32, kind="ExternalOutput").ap()

    with tile.TileContext(nc) as tc, ExitStack() as ctx:
        const = ctx.enter_context(tc.tile_pool(name="const", bufs=1))
        inp = ctx.enter_context(tc.tile_pool(name="inp", bufs=4))
        wpool = ctx.enter_context(tc.tile_pool(name="wpool", bufs=8))
        lpool = ctx.enter_context(tc.tile_pool(name="lpool", bufs=3))
        work = ctx.enter_context(tc.tile_pool(name="work", bufs=5))
        small = ctx.enter_context(tc.tile_pool(name="small", bufs=4))
        psum = ctx.enter_context(tc.tile_pool(name="psum", bufs=1, space="PSUM"))

        ones = const.tile([128, 1], f32)
        nc.vector.memset(ones[:], 1.0)
        bv_row = const.tile([1, L], f32)
        nc.scalar.dma_start(bv_row[:], bv_d[:])
        d_row = const.tile([1, L], f32)
        nc.scalar.dma_start(d_row[:], dd_d[:])
        bvb = const.tile([128, L], f32)
        nc.scalar.dma_start(bvb[:], bvb_d[:])
        ddb = const.tile([128, L], f32)
        nc.scalar.dma_start(ddb[:], ddb_d[:])

        flag_t = const.tile([128, n_tiles], f32)

        # PSUM accumulators: full colsums (PE route) + window tail colsums
        psq_a = psum.tile([1, 512], f32)
        psq_b = psum.tile([1, 512], f32)
        psl_a = psum.tile([1, 512], f32)
        psl_b = psum.tile([1, 512], f32)
        psw_q = psum.tile([1, W], f32)
        psw_l = psum.tile([1, W], f32)

        # DVE-route accumulators
        acc_main = const.tile([128, 1], f32)
        nc.vector.memset(acc_main[:], 0.0)
        acc_tail = const.tile([128, 1], f32)
        nc.vector.memset(acc_tail[:], 0.0)

        import os as _os
        _mode = _os.environ.get("DVE_PLACE", "spread")
        if _mode == "front":
            dve_set = set(range(n_dve))
        else:
            stride = max(1, n_tiles // max(n_dve, 1))
            dve_set = set((k * stride + stride - 1) % n_tiles for k in range(n_dve))
        n_pe_seen = 0
        assert n_tiles % 2 == 0
        pair_tiles = {}
        for i in range(n_tiles):
            if i % 2 == 0:
                r0 = i * 128
                out1_t2 = inp.tile([128, 2 * L], f32, tag="out1p")
                if i == 0:
                    nc.sync.dma_start(out1_t2[:, 0:L], out1_d[r0 : r0 + 128, :])
                    nc.sync.dma_start(out1_t2[:, L : 2 * L], out1_d[r0 + 128 : r0 + 256, :])
                else:
                    nc.sync.dma_start(
                        out1_t2[:].rearrange("p (two l) -> p two l", two=2),
                        out1_d[r0 : r0 + 256, :].rearrange("(two p) l -> p two l", p=128),
                    )
                wpack_t2 = wpool.tile([128, 4 * W], f32, tag="wpackp")
                nc.sync.dma_start(
                    wpack_t2[:].rearrange("p (two l) -> p two l", two=2),
                    wpack_d[r0 : r0 + 256, :].rearrange("(two p) l -> p two l", p=128),
                )
                lab_t2 = lpool.tile([128, 2 * L], u8, tag="labp")
                nc.scalar.dma_start(
                    lab_t2[:].rearrange("p (two l) -> p two l", two=2),
                    lab_d[r0 : r0 + 256, :].rearrange("(two p) l -> p two l", p=128),
                )
                pair_tiles = {"out1": out1_t2, "wpack": wpack_t2, "lab": lab_t2}
            half = i % 2
            dve_route = i in dve_set
            out1_t = pair_tiles["out1"][:, half * L : (half + 1) * L]
            lab_t = pair_tiles["lab"][:, half * L : (half + 1) * L]
            wp = pair_tiles["wpack"][:, half * 2 * W : (half + 1) * 2 * W]
            out0w_t = wp[:, 0:W]
            out1_w = wp[:, W : 2 * W]

            # ---- window mask: ge -> suffix-max s -> tail mask tm ----
            ge_w = work.tile([128, W], f16, tag="gew")
            nc.vector.tensor_tensor(ge_w[:], out0w_t, out1_w, Alu.is_ge)
            s_w = work.tile([128, W], f16, tag="sw")
            nc.vector.tensor_tensor_scan(
                s_w[:, ::-1], ge_w[:, ::-1], ge_w[:, ::-1], 0.0, Alu.max, Alu.max
            )
            # ao = 1 iff no zero decision inside the window (suspicious OR
            # genuinely all-ones row; either way tail contribution -> 0 and
            # the flag lets the host decide).
            nc.vector.tensor_scalar(
                flag_t[:, i : i + 1], s_w[:, 0:1], 0.0, None, Alu.is_equal
            )
            omao_col = small.tile([128, 1], f32, tag="omao")
            nc.vector.tensor_scalar(
                omao_col[:], flag_t[:, i : i + 1], -1.0, 1.0, Alu.mult, Alu.add
            )
            # tm = 1 - s - ao  (1 on the strict tail j > idx, else 0) on ScalarE
            tm_w = work.tile([128, W], f32, tag="tmw")
            nc.scalar.activation(
                tm_w[:], s_w[:], Act.Identity, bias=omao_col[:], scale=-1.0
            )

            if dve_route:
                # r1 = lab*D + Bv ; main = sum_j r1*out1 ; w kept for tail
                t1 = work.tile([128, L], f32, tag="t1")
                nc.vector.tensor_tensor(t1[:], lab_t, ddb[:], Alu.mult)
                r1 = work.tile([128, L], f32, tag="r1")
                nc.vector.tensor_tensor(r1[:], t1[:], bvb[:], Alu.add)
                wfull = work.tile([128, L], f32, tag="wfull")
                row_col = small.tile([128, 1], f32, tag="rowc")
                nc.vector.scalar_tensor_tensor(
                    wfull[:], r1[:], 1.0, out1_t, Alu.mult, Alu.mult,
                    accum_out=row_col[:],
                )
                nc.vector.tensor_tensor(acc_main[:], acc_main[:], row_col[:], Alu.add)
                # tail = sum_jw tm * w_window
                tail_col = small.tile([128, 1], f32, tag="tailc")
                junkw = work.tile([128, W], f32, tag="junkw")
                nc.vector.scalar_tensor_tensor(
                    junkw[:], tm_w[:], 1.0, wfull[:, L - W : L], Alu.mult, Alu.mult,
                    accum_out=tail_col[:],
                )
                nc.vector.tensor_tensor(acc_tail[:], acc_tail[:], tail_col[:], Alu.add)
            else:
                st, sp = n_pe_seen == 0, n_pe_seen == n_pe - 1
                n_pe_seen += 1
                # ql = out1 * lab
                ql = work.tile([128, L], f32, tag="ql")
                nc.vector.tensor_tensor(ql[:], out1_t, lab_t, Alu.mult)
                nc.tensor.matmul(psq_a[:], ones[:], out1_t[:, 0:512], start=st, stop=sp)
                nc.tensor.matmul(psq_b[:], ones[:], out1_t[:, 512:L], start=st, stop=sp)
                nc.tensor.matmul(psl_a[:], ones[:], ql[:, 0:512], start=st, stop=sp)
                nc.tensor.matmul(psl_b[:], ones[:], ql[:, 512:L], start=st, stop=sp)
                # tails: tail_q = tm*out1_w ; tail_ql = tail_q*lab_w
                tq = work.tile([128, W], f32, tag="tq")
                nc.vector.tensor_tensor(tq[:], tm_w[:], out1_w, Alu.mult)
                tl = work.tile([128, W], f32, tag="tl")
                nc.vector.tensor_tensor(tl[:], tq[:], lab_t[:, L - W : L], Alu.mult)
                nc.tensor.matmul(psw_q[:], ones[:], tq[:], start=st, stop=sp)
                nc.tensor.matmul(psw_l[:], ones[:], tl[:], start=st, stop=sp)

        # ---- epilogue: weighted dots straight out of PSUM ----
        res_t = const.tile([1, 8], f32)

        def dot(ps_ap, row_ap, k, tag):
            junk = const.tile([1, ps_ap.shape[1]], f32, tag="junk" + tag)
            nc.vector.scalar_tensor_tensor(
                junk[:], ps_ap, 1.0, row_ap, Alu.mult, Alu.mult,
                accum_out=res_t[0:1, k : k + 1],
            )

        dot(psq_a[:], bv_row[:, 0:512], 0, "1a")
        dot(psq_b[:], bv_row[:, 512:L], 1, "1b")
        dot(psl_a[:], d_row[:, 0:512], 2, "2a")
        dot(psl_b[:], d_row[:, 512:L], 3, "2b")
        dot(psw_q[:], bv_row[:, L - W : L], 4, "3")
        dot(psw_l[:], d_row[:, L - W : L], 5, "4")
        nc.vector.memset(res_t[0:1, 6:8], 0.0)
        nc.scalar.dma_start(res_d[:], res_t[:])
        nc.scalar.dma_start(accs_d[:, 0:1], acc_main[:])
        nc.scalar.dma_start(accs_d[:, 1:2], acc_tail[:])
        nc.scalar.dma_start(flag_d[:], flag_t[:])

    nc.compile()
    return nc


def _get_nc():
    if getattr(_compiled, "nc", None) is None:
        _compiled.nc = _build()
    return _compiled.nc


def _in_maps(output, labels):
    out1 = np.ascontiguousarray(output[:, :, 1], dtype=np.float32)
    wpack = np.empty((B, 2 * W), dtype=np.float32)
    wpack[:, 0:W] = output[:, L - W : L, 0]
    wpack[:, W : 2 * W] = output[:, L - W : L, 1]
    lab = labels.astype(np.uint8)  # values are 0/1
    bv, dd = _reward_rows()
    bvb = np.broadcast_to(bv, (128, L)).copy()
    ddb = np.broadcast_to(dd, (128, L)).copy()
    rp = ROWS_PER_CORE
    return [
        {
            "out1": out1[c * rp : (c + 1) * rp],
            "wpack": wpack[c * rp : (c + 1) * rp],
            "lab": lab[c * rp : (c + 1) * rp],
            "bv": bv.reshape(1, L),
            "dd": dd.reshape(1, L),
            "bvb": bvb,
            "ddb": ddb,
        }
        for c in range(N_CORES)
    ]


def _host_fallback(output, labels):
    temp = output[:, :, 1] > output[:, :, 0]
    allones = temp.all(axis=1)
    z = ~temp
    last_zero = (L - 1) - np.argmax(z[:, ::-1], axis=1)
    idx = np.where(allones, L, last_zero)
    mask = np.arange(L)[None, :] <= idx[:, None]
    j = np.arange(L, dtype=np.float64)
    r1 = np.where(labels == 1, -1.0 / np.log2(j + 2.0), (j + 1.0) / ALPHA)
    return np.float32(
        (output[:, :, 1].astype(np.float64) * mask * r1).sum() / B
    )


def _combine(results, output, labels):
    total = 0.0
    suspicious = 0.0
    for c, r in enumerate(results):
        p = np.asarray(r["partial"], dtype=np.float64)[0]
        total += p[0] + p[1] + p[2] + p[3] - p[4] - p[5]
        accs = np.asarray(r["accs"], dtype=np.float64)
        total += accs[:, 0].sum() - accs[:, 1].sum()
        # rows flagged "no zero in window": genuine all-ones rows are handled
        # (tail = 0) but a row whose last zero is before the window is not —
        # recheck on host. Never fires for +-symmetric random inputs.
        flags = np.asarray(r["flags"], dtype=np.float64)
        if flags.max() > 0:
            rp = ROWS_PER_CORE
            o = output[c * rp : (c + 1) * rp]
            allones_rows = (o[:, :, 1] > o[:, :, 0]).all(axis=1)
            flagged = flags.T.reshape(-1) > 0  # row-major within this core
            suspicious += (flagged & ~allones_rows).sum()
    if suspicious > 0:
        return _host_fallback(output, labels)
    return np.float32(total / B)


def kernel(output: np.ndarray, labels: np.ndarray) -> np.ndarray:
    from concourse.bass_utils import run_bass_kernel_spmd

    assert output.shape == (B, L, 2), output.shape
    nc = _get_nc()
    res = run_bass_kernel_spmd(
        nc, _in_maps(output, labels), core_ids=list(range(N_CORES))
    )
    return _combine(res.results, output, labels)



# revision 5
# speedup vs baseline: 1.5087x; 1.5087x over previous
"""BiCutLoss Trainium2 kernel (8-core data parallel over batch).

Reference semantics (B=16384, L=1024):
    temp[b,j]  = argmax(output[b,j,:])          # 1 iff out1 > out0 (ties -> 0)
    idx[b]     = L if row all-ones else index of last zero
    mask[b,j]  = j <= idx[b]
    r1[b,j]    = -1/log2(j+2)  if labels==1 else (j+1)/alpha
    loss       = sum(output[...,1] * mask * r1) / B

Restructuring: masked_sum = full_sum - tail_sum; the tail (j > idx) is
confined to the last W columns whenever each row has a zero decision in
its last W positions (P(violation) = 2^-W per random row; flags catch it
and the host falls back to an exact evaluation).

v2 design (memory-regime):
  - All value data f16 (halves HBM traffic; DVE 2x mode; PE 1 cyc/row).
  - Labels travel as u8 (2MB/core) and are converted u8->f16 on ScalarE
    (otherwise idle), so ql = out1*lab runs at DVE 2x.
  - Column sums via PE matmuls with one-hot [128,4] stationaries into a
    single PSUM tile [4,512] per chain: row0 = colsum(out1) j<512,
    row1 = colsum(out1) j>=512, row2/3 = same for ql. The window tail
    accumulates NEGATED into rows 1/3 cols 448:512 (same j columns), so
    no extra accumulators or dots are needed.
  - Epilogue per chain: one scalar_tensor_tensor over [4,512] PSUM with
    the [4,512] weight rows (Bv lo/hi, D lo/hi) and per-partition
    accum_out -> 4 partial dot products. Host sums them.
  - Window mask ops (ge/scan/neg_tq/neg_tl) on [128,64] row-layout
    slices, split between DVE and GpSimd.
"""

import os
import threading
from contextlib import ExitStack

import numpy as np

B, L = 16384, 1024
N_CORES = 8
ROWS_PER_CORE = B // N_CORES  # 2048
N_TILES = ROWS_PER_CORE // 128  # 16
ALPHA = 0.65
W = 32  # tail window width

# PSUM accumulation chains (tile counts); small last group shortens tail.
CHAIN_GROUPS = (5, 5, 4, 2)

_compiled = threading.local()


def _reward_rows():
    j = np.arange(L, dtype=np.float64)
    bv = (j + 1.0) / ALPHA
    d = -1.0 / np.log2(j + 2.0) - bv
    return bv.astype(np.float32), d.astype(np.float32)


def _build(rows=ROWS_PER_CORE, num_devices=N_CORES):
    import concourse.tile as tile
    from concourse import bacc, mybir

    f32 = mybir.dt.float32
    f16 = mybir.dt.float16
    u8 = mybir.dt.uint8
    Alu = mybir.AluOpType

    n_tiles = rows // 128
    assert n_tiles == N_TILES and n_tiles % 2 == 0
    n_chains = len(CHAIN_GROUPS)

    # engine placement toggles (A/B-able via env)
    conv_eng = os.environ.get("CONV_ENG", "act")

    nc = bacc.Bacc(
        "TRN2",
        target_bir_lowering=False,
        debug=False,
        enable_asserts=True,
        num_devices=num_devices,
    )

    def eng(name):
        return {"dve": nc.vector, "gp": nc.gpsimd, "act": nc.scalar}[name]

    out1_d = nc.dram_tensor("out1", [rows, L], f16, kind="ExternalInput").ap()
    lab_d = nc.dram_tensor("lab", [rows, L], u8, kind="ExternalInput").ap()
    # out0 window, host-packed partition-major: [128, n_tiles*W]
    w0_d = nc.dram_tensor("w0", [128, n_tiles * W], f16, kind="ExternalInput").ap()
    # dot weights: rows = [Bv_lo, Bv_hi, D_lo, D_hi]
    wrow_d = nc.dram_tensor("wrow", [4, 512], f32, kind="ExternalInput").ap()
    # output: cols 0:n_tiles = flags, col n_tiles+c (partitions 0:4) = chain dots
    res_d = nc.dram_tensor(
        "res", [128, n_tiles + n_chains], f32, kind="ExternalOutput"
    ).ap()

    with tile.TileContext(nc) as tc, ExitStack() as ctx:
        const = ctx.enter_context(tc.tile_pool(name="const", bufs=1))
        inp = ctx.enter_context(tc.tile_pool(name="inp", bufs=3))
        labp = ctx.enter_context(tc.tile_pool(name="labp", bufs=3))
        labf_p = ctx.enter_context(tc.tile_pool(name="labf", bufs=3))
        work = ctx.enter_context(tc.tile_pool(name="work", bufs=4))
        win = ctx.enter_context(tc.tile_pool(name="win", bufs=4))
        psum = ctx.enter_context(tc.tile_pool(name="psum", bufs=1, space="PSUM"))

        # ---- constants ----
        w0_t = const.tile([128, n_tiles * W], f16)
        nc.sync.dma_start(w0_t[:], w0_d[:])
        wrow_t = const.tile([4, 512], f32)
        nc.sync.dma_start(wrow_t[:], wrow_d[:])
        # one-hot stationaries e_c [128,4]: column c all-ones
        e_st = []
        for c in range(4):
            e = const.tile([128, 4], f16, tag=f"e{c}")
            nc.vector.memset(e[:], 0.0)
            nc.vector.memset(e[:, c : c + 1], 1.0)
            e_st.append(e)

        res_t = const.tile([128, n_tiles + n_chains], f32)
        flag_t = res_t[:, 0:n_tiles]

        # PSUM accumulators: one [4,512] tile (one bank) per chain
        ps = [
            psum.tile([4, 512], f32, tag=f"ps{c}", name=f"ps{c}")
            for c in range(n_chains)
        ]

        # chain id per tile
        chain_of = []
        for c, g in enumerate(CHAIN_GROUPS):
            chain_of += [c] * g
        assert len(chain_of) == n_tiles
        chain_start = {}
        chain_end = {}
        for i, c in enumerate(chain_of):
            chain_start.setdefault(c, i)
            chain_end[c] = i

        pair = {}
        for i in range(n_tiles):
            if i % 2 == 0:
                # ---- pair DMA loads ----
                r0 = i * 128
                o_t = inp.tile([128, 2 * L], f16, tag="o1p")
                nc.sync.dma_start(
                    o_t[:].rearrange("p (g l) -> p g l", g=2),
                    out1_d[r0 : r0 + 256, :].rearrange("(g p) l -> p g l", p=128),
                )
                l_t = labp.tile([128, 2 * L], u8, tag="labp")
                nc.sync.dma_start(
                    l_t[:].rearrange("p (g l) -> p g l", g=2),
                    lab_d[r0 : r0 + 256, :].rearrange("(g p) l -> p g l", p=128),
                )
                # label conversion u8 -> f16 for the pair
                lf_t = labf_p.tile([128, 2 * L], f16, tag="labf")
                if conv_eng == "act":
                    nc.scalar.copy(lf_t[:], l_t[:])
                elif conv_eng == "gp":
                    nc.gpsimd.tensor_copy(lf_t[:], l_t[:])
                else:
                    nc.vector.tensor_copy(lf_t[:], l_t[:])
                s2 = win.tile([128, 2 * W], f16, tag="spair")
                pair = {"o": o_t, "lf": lf_t, "s": s2}

            half = i % 2
            c = chain_of[i]
            sp = i == chain_end[c]
            out1_t = pair["o"][:, half * L : (half + 1) * L]
            labf_t = pair["lf"][:, half * L : (half + 1) * L]

            # ---- ql = out1 * labf (DVE 2x) ----
            ql = work.tile([128, L], f16, tag="ql")
            nc.vector.tensor_tensor(ql[:], out1_t, labf_t, Alu.mult)

            # ---- window pipeline ----
            out1_w = out1_t[:, L - W : L]
            out0_w = w0_t[:, i * W : (i + 1) * W]
            ge_w = win.tile([128, W], f16, tag="ge")
            nc.vector.tensor_tensor(ge_w[:], out0_w, out1_w, Alu.is_ge)
            s_w = pair["s"][:, half * W : (half + 1) * W]
            nc.vector.tensor_tensor_scan(
                s_w[:, ::-1], ge_w[:, ::-1], ge_w[:, ::-1], 0.0, Alu.max, Alu.max
            )
            if half == 1:
                # flags for the pair: flag = (s[0] == 0), 1 iff no
                # zero-decision in the window (suspicious or all-ones row)
                nc.vector.tensor_scalar(
                    flag_t[:, i - 1 : i + 1],
                    pair["s"][:, 0 : 2 * W : W],
                    0.0,
                    None,
                    Alu.is_equal,
                )
            # neg_tq = (s - s[0]) * out1_w  ( = -(strict tail mask) * out1_w;
            # s[0] = 1 - allones_flag, so suspicious rows contribute 0 )
            ntq = win.tile([128, W], f16, tag="ntq")
            nc.vector.scalar_tensor_tensor(
                ntq[:],
                s_w,
                s_w[:, 0:1],
                out1_w,
                Alu.subtract,
                Alu.mult,
            )
            ntl = win.tile([128, W], f16, tag="ntl")
            nc.vector.tensor_tensor(ntl[:], ntq[:], labf_t[:, L - W : L], Alu.mult)

            # ---- matmuls into this chain's accumulator ----
            pst = ps[c]
            st = i == chain_start[c]
            nc.tensor.matmul(pst[:], e_st[0][:], out1_t[:, 0:512], start=st, stop=False)
            nc.tensor.matmul(pst[:], e_st[1][:], out1_t[:, 512:L], start=False, stop=False)
            nc.tensor.matmul(pst[:], e_st[2][:], ql[:, 0:512], start=False, stop=False)
            nc.tensor.matmul(pst[:], e_st[3][:], ql[:, 512:L], start=False, stop=False)
            # window tails, negated, land on cols 512-W:512 (j in [L-W, L))
            nc.tensor.matmul(
                pst[0:4, 512 - W : 512], e_st[1][:], ntq[:], start=False, stop=False
            )
            nc.tensor.matmul(
                pst[0:4, 512 - W : 512], e_st[3][:], ntl[:], start=False, stop=sp
            )

            if sp:
                junk = work.tile([4, 512], f32, tag=f"junk{c}")
                nc.vector.scalar_tensor_tensor(
                    junk[:],
                    pst[:],
                    1.0,
                    wrow_t[:],
                    Alu.mult,
                    Alu.mult,
                    accum_out=res_t[0:4, n_tiles + c : n_tiles + c + 1],
                )

        nc.scalar.dma_start(res_d[:], res_t[:])

    nc.compile()
    return nc


def _get_nc():
    if getattr(_compiled, "nc", None) is None:
        _compiled.nc = _build()
    return _compiled.nc


def _in_maps(output, labels):
    out1 = np.ascontiguousarray(output[:, :, 1]).astype(np.float16)
    lab = labels.astype(np.uint8)
    out0w = np.ascontiguousarray(output[:, L - W :, 0]).astype(np.float16)
    bv, dd = _reward_rows()
    wrow = np.stack([bv[0:512], bv[512:L], dd[0:512], dd[512:L]]).astype(np.float32)
    rp = ROWS_PER_CORE
    maps = []
    for c in range(N_CORES):
        # pack out0 window partition-major: [128, n_tiles*W]
        w0c = (
            out0w[c * rp : (c + 1) * rp]
            .reshape(N_TILES, 128, W)
            .transpose(1, 0, 2)
            .reshape(128, N_TILES * W)
        )
        maps.append(
            {
                "out1": out1[c * rp : (c + 1) * rp],
                "lab": lab[c * rp : (c + 1) * rp],
                "w0": np.ascontiguousarray(w0c),
                "wrow": wrow,
            }
        )
    return maps


def _host_fallback(output, labels):
    temp = output[:, :, 1] > output[:, :, 0]
    allones = temp.all(axis=1)
    z = ~temp
    last_zero = (L - 1) - np.argmax(z[:, ::-1], axis=1)
    idx = np.where(allones, L, last_zero)
    mask = np.arange(L)[None, :] <= idx[:, None]
    j = np.arange(L, dtype=np.float64)
    r1 = np.where(labels == 1, -1.0 / np.log2(j + 2.0), (j + 1.0) / ALPHA)
    return np.float32((output[:, :, 1].astype(np.float64) * mask * r1).sum() / B)


def _combine(results, output, labels):
    total = 0.0
    suspicious = 0
    n_chains = len(CHAIN_GROUPS)
    for c, r in enumerate(results):
        res = np.asarray(r["res"], dtype=np.float64)
        total += res[0:4, N_TILES : N_TILES + n_chains].sum()
        flags = res[:, 0:N_TILES]
        if flags.max() > 0:
            rp = ROWS_PER_CORE
            o = output[c * rp : (c + 1) * rp]
            allones_rows = (o[:, :, 1] > o[:, :, 0]).all(axis=1)
            flagged = flags.T.reshape(-1) > 0  # row-major within this core
            suspicious += int((flagged & ~allones_rows).sum())
    if suspicious > 0:
        return _host_fallback(output, labels)
    return np.float32(total / B)


def kernel(output: np.ndarray, labels: np.ndarray) -> np.ndarray:
    from concourse.bass_utils import run_bass_kernel_spmd

    assert output.shape == (B, L, 2), output.shape
    nc = _get_nc()
    res = run_bass_kernel_spmd(
        nc, _in_maps(output, labels), core_ids=list(range(N_CORES))
    )
    return _combine(res.results, output, labels)


# revision 6
# speedup vs baseline: 1.5739x; 1.0432x over previous
"""BiCutLoss Trainium2 kernel (8-core data parallel over batch).

Reference semantics (B=16384, L=1024):
    temp[b,j]  = argmax(output[b,j,:])          # 1 iff out1 > out0 (ties -> 0)
    idx[b]     = L if row all-ones else index of last zero
    mask[b,j]  = j <= idx[b]
    r1[b,j]    = -1/log2(j+2)  if labels==1 else (j+1)/alpha
    loss       = sum(output[...,1] * mask * r1) / B

Restructuring: masked_sum = full_sum - tail_sum; the tail (j > idx) is
confined to the last W columns whenever each row has a zero decision in
its last W positions (P(violation) = 2^-W per random row; flags catch it
and the host falls back to an exact evaluation).

v2 design (memory-regime):
  - All value data f16 (halves HBM traffic; DVE 2x mode; PE 1 cyc/row).
  - Labels travel as u8 (2MB/core) and are converted u8->f16 on ScalarE
    (otherwise idle), so ql = out1*lab runs at DVE 2x.
  - Column sums via PE matmuls with one-hot [128,4] stationaries into a
    single PSUM tile [4,512] per chain: row0 = colsum(out1) j<512,
    row1 = colsum(out1) j>=512, row2/3 = same for ql. The window tail
    accumulates NEGATED into rows 1/3 cols 448:512 (same j columns), so
    no extra accumulators or dots are needed.
  - Epilogue per chain: one scalar_tensor_tensor over [4,512] PSUM with
    the [4,512] weight rows (Bv lo/hi, D lo/hi) and per-partition
    accum_out -> 4 partial dot products. Host sums them.
  - Window mask ops (ge/scan/neg_tq/neg_tl) on [128,64] row-layout
    slices, split between DVE and GpSimd.
"""

import os
import threading
from contextlib import ExitStack

import numpy as np

B, L = 16384, 1024
N_CORES = 8
ROWS_PER_CORE = B // N_CORES  # 2048
N_TILES = ROWS_PER_CORE // 128  # 16
ALPHA = 0.65
W = 32  # tail window width

# PSUM accumulation chains (tile counts); small last group shortens tail.
CHAIN_GROUPS = (5, 5, 4, 2)

_compiled = threading.local()


def _reward_rows():
    j = np.arange(L, dtype=np.float64)
    bv = (j + 1.0) / ALPHA
    d = -1.0 / np.log2(j + 2.0) - bv
    return bv.astype(np.float32), d.astype(np.float32)


def _build(rows=ROWS_PER_CORE, num_devices=N_CORES):
    import concourse.tile as tile
    from concourse import bacc, mybir

    f32 = mybir.dt.float32
    f16 = mybir.dt.float16
    u8 = mybir.dt.uint8
    Alu = mybir.AluOpType

    n_tiles = rows // 128
    assert n_tiles == N_TILES and n_tiles % 2 == 0
    n_chains = len(CHAIN_GROUPS)

    # engine placement toggles (A/B-able via env)
    conv_eng = os.environ.get("CONV_ENG", "act")

    nc = bacc.Bacc(
        "TRN2",
        target_bir_lowering=False,
        debug=False,
        enable_asserts=True,
        num_devices=num_devices,
    )

    def eng(name):
        return {"dve": nc.vector, "gp": nc.gpsimd, "act": nc.scalar}[name]

    out1_d = nc.dram_tensor("out1", [rows, L], f16, kind="ExternalInput").ap()
    lab_d = nc.dram_tensor("lab", [rows, L], u8, kind="ExternalInput").ap()
    # out0 window, host-packed partition-major: [128, n_tiles*W]
    w0_d = nc.dram_tensor("w0", [128, n_tiles * W], f16, kind="ExternalInput").ap()
    # dot weights: rows = [Bv_lo, Bv_hi, D_lo, D_hi]
    wrow_d = nc.dram_tensor("wrow", [4, 512], f32, kind="ExternalInput").ap()
    # output: cols 0:n_tiles = flags, col n_tiles+c (partitions 0:4) = chain dots
    res_d = nc.dram_tensor(
        "res", [128, n_tiles + n_chains], f32, kind="ExternalOutput"
    ).ap()

    with tile.TileContext(nc) as tc, ExitStack() as ctx:
        const = ctx.enter_context(tc.tile_pool(name="const", bufs=1))
        inp = ctx.enter_context(tc.tile_pool(name="inp", bufs=4))
        labp = ctx.enter_context(tc.tile_pool(name="labp", bufs=4))
        labf_p = ctx.enter_context(tc.tile_pool(name="labf", bufs=8))
        work = ctx.enter_context(tc.tile_pool(name="work", bufs=4))
        win = ctx.enter_context(tc.tile_pool(name="win", bufs=4))
        wint = ctx.enter_context(tc.tile_pool(name="wint", bufs=8))
        psum = ctx.enter_context(tc.tile_pool(name="psum", bufs=1, space="PSUM"))

        # ---- constants ----
        w0_t = const.tile([128, n_tiles * W], f16)
        nc.sync.dma_start(w0_t[:], w0_d[:])
        wrow_t = const.tile([4, 512], f32)
        nc.sync.dma_start(wrow_t[:], wrow_d[:])
        # one-hot stationaries e_c [128,4]: column c all-ones
        e_st = []
        for c in range(4):
            e = const.tile([128, 4], f16, tag=f"e{c}")
            nc.vector.memset(e[:], 0.0)
            nc.vector.memset(e[:, c : c + 1], 1.0)
            e_st.append(e)

        res_t = const.tile([128, n_tiles + n_chains], f32)
        flag_t = res_t[:, 0:n_tiles]

        # PSUM accumulators: one [4,512] tile (one bank) per chain
        ps = [
            psum.tile([4, 512], f32, tag=f"ps{c}", name=f"ps{c}")
            for c in range(n_chains)
        ]

        # chain id per tile
        chain_of = []
        for c, g in enumerate(CHAIN_GROUPS):
            chain_of += [c] * g
        assert len(chain_of) == n_tiles
        chain_start = {}
        chain_end = {}
        for i, c in enumerate(chain_of):
            chain_start.setdefault(c, i)
            chain_end[c] = i

        # deferred window matmuls: per chain, list of (e_idx, moving_ap)
        pending = {c: [] for c in range(n_chains)}

        def flush_chain(c):
            items = pending[c]
            for k, (ei, mov) in enumerate(items):
                nc.tensor.matmul(
                    ps[c][0:4, 512 - W : 512],
                    e_st[ei][:],
                    mov,
                    start=False,
                    stop=k == len(items) - 1,
                )
            pending[c] = None
            junk = work.tile([4, 512], f32, tag=f"junk{c}", name=f"junk{c}")
            nc.vector.scalar_tensor_tensor(
                junk[:],
                ps[c][:],
                1.0,
                wrow_t[:],
                Alu.mult,
                Alu.mult,
                accum_out=res_t[0:4, n_tiles + c : n_tiles + c + 1],
            )

        quad = {}
        pair = {}
        for i in range(n_tiles):
            if i % 4 == 0:
                # ---- quad DMA loads (1MB f16 + 512KB u8) ----
                r0 = i * 128
                o_t = inp.tile([128, 4 * L], f16, tag="o1q")
                nc.sync.dma_start(
                    o_t[:].rearrange("p (g l) -> p g l", g=4),
                    out1_d[r0 : r0 + 512, :].rearrange("(g p) l -> p g l", p=128),
                )
                l_t = labp.tile([128, 4 * L], u8, tag="labq")
                nc.gpsimd.dma_start(
                    l_t[:].rearrange("p (g l) -> p g l", g=4),
                    lab_d[r0 : r0 + 512, :].rearrange("(g p) l -> p g l", p=128),
                )
                quad = {"o": o_t, "l": l_t}
            if i % 2 == 0:
                q2 = (i % 4) // 2
                # label conversion u8 -> f16 for the pair
                lf_t = labf_p.tile([128, 2 * L], f16, tag="labf")
                if conv_eng == "act":
                    nc.scalar.copy(lf_t[:], quad["l"][:, q2 * 2 * L : (q2 + 1) * 2 * L])
                elif conv_eng == "gp":
                    nc.gpsimd.tensor_copy(
                        lf_t[:], quad["l"][:, q2 * 2 * L : (q2 + 1) * 2 * L]
                    )
                else:
                    nc.vector.tensor_copy(
                        lf_t[:], quad["l"][:, q2 * 2 * L : (q2 + 1) * 2 * L]
                    )
                s2 = win.tile([128, 2 * W], f16, tag="spair")
                pair = {"lf": lf_t, "s": s2}

            half = i % 2
            qi = i % 4
            c = chain_of[i]
            out1_t = quad["o"][:, qi * L : (qi + 1) * L]
            labu_t = quad["l"][:, qi * L : (qi + 1) * L]
            labf_t = pair["lf"][:, half * L : (half + 1) * L]

            # ---- ql = out1 * labf (DVE 2x) ----
            ql = work.tile([128, L], f16, tag="ql")
            nc.vector.tensor_tensor(ql[:], out1_t, labf_t, Alu.mult)

            # ---- window pipeline ----
            out1_w = out1_t[:, L - W : L]
            out0_w = w0_t[:, i * W : (i + 1) * W]
            ge_w = win.tile([128, W], f16, tag="ge")
            nc.vector.tensor_tensor(ge_w[:], out0_w, out1_w, Alu.is_ge)
            s_w = pair["s"][:, half * W : (half + 1) * W]
            nc.vector.tensor_tensor_scan(
                s_w[:, ::-1], ge_w[:, ::-1], ge_w[:, ::-1], 0.0, Alu.max, Alu.max
            )
            if half == 1:
                # flags for the pair: flag = (s[0] == 0), 1 iff no
                # zero-decision in the window (suspicious or all-ones row)
                nc.vector.tensor_scalar(
                    flag_t[:, i - 1 : i + 1],
                    pair["s"][:, 0 : 2 * W : W],
                    0.0,
                    None,
                    Alu.is_equal,
                )
            # neg_tq = (s - s[0]) * out1_w  ( = -(strict tail mask) * out1_w;
            # s[0] = 1 - allones_flag, so suspicious rows contribute 0 )
            ntq = wint.tile([128, W], f16, tag="ntq")
            nc.vector.scalar_tensor_tensor(
                ntq[:],
                s_w,
                s_w[:, 0:1],
                out1_w,
                Alu.subtract,
                Alu.mult,
            )
            ntl = wint.tile([128, W], f16, tag="ntl")
            nc.vector.tensor_tensor(ntl[:], ntq[:], labu_t[:, L - W : L], Alu.mult)

            # ---- main matmuls into this chain's accumulator ----
            pst = ps[c]
            st = i == chain_start[c]
            nc.tensor.matmul(pst[:], e_st[0][:], out1_t[:, 0:512], start=st, stop=False)
            nc.tensor.matmul(pst[:], e_st[1][:], out1_t[:, 512:L], start=False, stop=False)
            nc.tensor.matmul(pst[:], e_st[2][:], ql[:, 0:512], start=False, stop=False)
            nc.tensor.matmul(pst[:], e_st[3][:], ql[:, 512:L], start=False, stop=False)
            # window tails (negated, cols 512-W:512) are deferred two tiles so
            # the PE never waits on the window scan chain
            pending[c].append((1, ntq[:]))
            pending[c].append((3, ntl[:]))
            if i - 2 >= 0 and chain_end[chain_of[i - 2]] == i - 2:
                flush_chain(chain_of[i - 2])

        for c in range(n_chains):
            if pending[c] is not None:
                flush_chain(c)

        nc.scalar.dma_start(res_d[:], res_t[:])

    nc.compile()
    return nc


def _get_nc():
    if getattr(_compiled, "nc", None) is None:
        _compiled.nc = _build()
    return _compiled.nc


def _in_maps(output, labels):
    out1 = np.ascontiguousarray(output[:, :, 1]).astype(np.float16)
    lab = labels.astype(np.uint8)
    out0w = np.ascontiguousarray(output[:, L - W :, 0]).astype(np.float16)
    bv, dd = _reward_rows()
    wrow = np.stack([bv[0:512], bv[512:L], dd[0:512], dd[512:L]]).astype(np.float32)
    rp = ROWS_PER_CORE
    maps = []
    for c in range(N_CORES):
        # pack out0 window partition-major: [128, n_tiles*W]
        w0c = (
            out0w[c * rp : (c + 1) * rp]
            .reshape(N_TILES, 128, W)
            .transpose(1, 0, 2)
            .reshape(128, N_TILES * W)
        )
        maps.append(
            {
                "out1": out1[c * rp : (c + 1) * rp],
                "lab": lab[c * rp : (c + 1) * rp],
                "w0": np.ascontiguousarray(w0c),
                "wrow": wrow,
            }
        )
    return maps


def _host_fallback(output, labels):
    temp = output[:, :, 1] > output[:, :, 0]
    allones = temp.all(axis=1)
    z = ~temp
    last_zero = (L - 1) - np.argmax(z[:, ::-1], axis=1)
    idx = np.where(allones, L, last_zero)
    mask = np.arange(L)[None, :] <= idx[:, None]
    j = np.arange(L, dtype=np.float64)
    r1 = np.where(labels == 1, -1.0 / np.log2(j + 2.0), (j + 1.0) / ALPHA)
    return np.float32((output[:, :, 1].astype(np.float64) * mask * r1).sum() / B)


def _combine(results, output, labels):
    total = 0.0
    suspicious = 0
    n_chains = len(CHAIN_GROUPS)
    for c, r in enumerate(results):
        res = np.asarray(r["res"], dtype=np.float64)
        total += res[0:4, N_TILES : N_TILES + n_chains].sum()
        flags = res[:, 0:N_TILES]
        if flags.max() > 0:
            rp = ROWS_PER_CORE
            o = output[c * rp : (c + 1) * rp]
            allones_rows = (o[:, :, 1] > o[:, :, 0]).all(axis=1)
            flagged = flags.T.reshape(-1) > 0  # row-major within this core
            suspicious += int((flagged & ~allones_rows).sum())
    if suspicious > 0:
        return _host_fallback(output, labels)
    return np.float32(total / B)


def kernel(output: np.ndarray, labels: np.ndarray) -> np.ndarray:
    from concourse.bass_utils import run_bass_kernel_spmd

    assert output.shape == (B, L, 2), output.shape
    nc = _get_nc()
    res = run_bass_kernel_spmd(
        nc, _in_maps(output, labels), core_ids=list(range(N_CORES))
    )
    return _combine(res.results, output, labels)
